# revision 36
# baseline (speedup 1.0000x reference)
"""Dynamic-masked linear (topk_masking) on 8 TRN2 NeuronCores.

Computes reference:
    idx = nonzero(mask)            # exactly K=8192 of 16384
    out = data @ weight[idx].T + bias[idx]     # [8192 tok, 8192 sel]

Strategy (data-parallel over tokens, full selected-weight replicated):
  * Host: nonzero + row-gather of weight/bias (cheap vs 550 GFLOP matmul),
    pack operands into DMA-friendly layouts (partition-major contiguous).
  * Each core m computes out^T[:, m*1024:(m+1)*1024] = W_sel @ X_m^T
    as a PE-stationary-weight matmul over 32 contraction blocks of 128.
  * Mixed precision on the contraction: 28 blocks run as fp16 matmuls
    (1 col/cycle), the last 4 blocks run as 2 fp8-e4m3 DoubleRow matmuls
    (contraction 256 per instr at 1 col/cycle = 2x fp16 throughput,
    measured 220 ns/instr either way). PSUM accumulates fp32. Measured
    full-size rel err ~1.5e-2 (fp8 part dominates; tolerance 2e-2) vs
    3e-4 for pure fp16. 30 instead of 32 PE instrs per output panel.
  * Bias added during PSUM->SBUF eviction via DVE tensor_scalar_add
    (bias is per-partition in the out^T layout).
  * Host: concat the 8 token-slices of out^T, transpose once.

Per-core: 68.7 GFLOP; PE-bound. HBM ~95 MiB (hidden under compute).
"""

import contextlib
import sys
import types
from collections import Counter

import ml_dtypes
import numpy as np

import concourse.bacc as bacc
import concourse.bass as bass
import concourse.mybir as mybir
import concourse.tile as tile
from concourse.bass_utils import run_bass_kernel_spmd


def _ensure_axon_hooks():
    """run_bass_kernel_spmd imports antenv.axon_hooks when tracing is
    requested (e.g. BASS_TRACE=1). Some agent images lack that module;
    provide the real ctypes-based hook when possible, else a None hook so
    tracing degrades gracefully instead of crashing the kernel."""
    if "antenv.axon_hooks" in sys.modules:
        return
    try:
        import antenv.axon_hooks  # noqa: F401
        return
    except ImportError:
        pass
    hook = None
    try:
        from trn_agent_boot.trn_boot import _ntff_profile_via_ctypes
        hook = _ntff_profile_via_ctypes("/opt/axon/libaxon_pjrt.so")
    except Exception:
        pass
    mod = types.ModuleType("antenv.axon_hooks")
    mod.get_axon_ntff_profile_hook = lambda: hook
    mod.set_axon_ntff_profile_hook = lambda h: None
    sys.modules["antenv.axon_hooks"] = mod


_ensure_axon_hooks()

N_CORES = 8
P = 128

# Full-problem dims (hardcoded per harness contract)
IN_F = 4096
OUT_F = 16384
N_TOK = 8192
K_SEL = OUT_F // 2
TOK_PER_CORE = N_TOK // N_CORES  # 1024

IB_N = IN_F // P    # 32 contraction blocks
JB_N = K_SEL // P   # 64 output-column panels
TB_SIZE = 512       # moving free dim per matmul (one PSUM bank of fp32)

# Contraction split: first IB16_N blocks in fp16, last 4 in fp8 e4m3
# (2 DoubleRow pairs). More fp8 blows the 2e-2 error budget: full-fp8
# rel err is 4.4e-2 and scales with sqrt(n_fp8/32).
FP8_PAIRS = 2
IB8_N = 2 * FP8_PAIRS
IB16_N = IB_N - IB8_N

F32 = mybir.dt.float32
F16 = mybir.dt.float16
F8 = mybir.dt.float8e4
NP8 = ml_dtypes.float8_e4m3


def build_program(ib16_n=IB16_N, pairs=FP8_PAIRS, toks=TOK_PER_CORE,
                  tb_size=TB_SIZE, jb_n=JB_N, w_bufs=6):
    """Build the per-core Bass program.

    DRAM parameter layouts (host packs these):
      wt16 [jb_n, P, ib16_n, P]  : wt16[jb, p, a, c] = f16(W_sel[jb*P+c, a*P+p])
      wt8  [jb_n, P, pairs, 2, P]: wt8[jb, p, d, i, c]
                                     = e4m3(W_sel[jb*P+c, (ib16_n+2d+i)*P+p])
      xt16 [tb_n, P, ib16_n, tb] : xt16[tb, p, a, t]
                                     = f16(data[tok0+tb*tb_size+t, a*P+p])
      xt8  [P, pairs, 2, toks]   : xt8[p, d, i, t]
                                     = e4m3(data[tok0+t, (ib16_n+2d+i)*P+p])
      bs   [P, jb_n]             : bs[c, jb] = b_sel[jb*P + c]
      out  [jb_n, P, toks]       : out[jb, c, t] = out^T[jb*P+c, tok0+t]
    """
    tb_n = toks // tb_size
    assert toks % tb_size == 0

    nc = bacc.Bacc(
        "TRN2", target_bir_lowering=False, debug=False, num_devices=N_CORES
    )
    wt16 = nc.declare_dram_parameter(
        "wt16", [jb_n, P, ib16_n, P], F16, isOutput=False)
    xt16 = nc.declare_dram_parameter(
        "xt16", [tb_n, P, ib16_n, tb_size], F16, isOutput=False)
    if pairs:
        wt8 = nc.declare_dram_parameter(
            "wt8", [jb_n, P, pairs, 2, P], F8, isOutput=False)
        xt8 = nc.declare_dram_parameter(
            "xt8", [tb_n, P, pairs, 2, tb_size], F8, isOutput=False)
    bs = nc.declare_dram_parameter("bs", [P, jb_n], F32, isOutput=False)
    out = nc.declare_dram_parameter("out", [jb_n, P, toks], F32, isOutput=True)

    # x16-load chunking: few DMA triggers (issue is ~650 ns each per queue),
    # ramping sizes so the first matmuls unblock as early as possible.
    xc_sizes = [1, 1, 2, 4]
    while sum(xc_sizes) < ib16_n:
        xc_sizes.append(min(8, ib16_n - sum(xc_sizes)))
    xc_start = np.cumsum([0] + xc_sizes)
    ib2chunk = {}
    for c, (st, sz) in enumerate(zip(xc_start, xc_sizes)):
        for k in range(sz):
            ib2chunk[st + k] = (c, k)

    size_counts = Counter(xc_sizes)

    with tile.TileContext(nc) as tc:
        with contextlib.ExitStack() as stk:
            xpools = {
                sz: stk.enter_context(
                    tc.tile_pool(name=f"xpool{sz}", bufs=cnt * tb_n))
                for sz, cnt in size_counts.items()
            }
            wpool = stk.enter_context(tc.tile_pool(name="wpool", bufs=w_bufs))
            if pairs:
                x8pool = stk.enter_context(
                    tc.tile_pool(name="x8pool", bufs=tb_n))
                w8pool = stk.enter_context(
                    tc.tile_pool(name="w8pool", bufs=w_bufs))
            bpool = stk.enter_context(tc.tile_pool(name="bpool", bufs=1))
            opool = stk.enter_context(tc.tile_pool(name="opool", bufs=6))
            pspool = stk.enter_context(
                tc.tile_pool(name="pspool", bufs=8, space="PSUM"))

            # Startup DMAs ride three engine queues as dedicated streams,
            # each stream ordered by consumption time. A queue issues its
            # DMAs in order (ring-blocking preserves order), so streams
            # deliver bytes roughly when the warm phase needs them:
            #   sync   — X stream: x8, then x16 chunks tb0 then tb1
            #   gpsimd — warm-W stream: bias, w8, w16 of warm panels
            #   scalar — steady prefetch: W panels k_warm..w_prefetch-1
            k_warm = min(4, jb_n)
            w_prefetch = min(w_bufs, jb_n)
            w_tiles = []
            w8_tiles = []

            b_sb = bpool.tile([P, jb_n], F32)
            nc.gpsimd.dma_start(out=b_sb[:], in_=bs[:])
            x8_sbs = []
            if pairs:
                for tb in range(tb_n):
                    x8_sb = x8pool.tile(
                        [P, pairs, 2, tb_size], F8, name="x8_sb")
                    (nc.sync if tb % 2 == 0 else nc.scalar).dma_start(
                        out=x8_sb[:], in_=xt8[tb])
                    x8_sbs.append(x8_sb)
                for jb in range(k_warm):
                    w8_sb = w8pool.tile([P, pairs, 2, P], F8, name="w8_sb")
                    nc.gpsimd.dma_start(out=w8_sb[:], in_=wt8[jb])
                    w8_tiles.append(w8_sb)

            x_chunks = {}
            xq = [0]

            def x_issue(c, tb):
                st, sz = xc_start[c], xc_sizes[c]
                x_sb = xpools[sz].tile([P, sz, tb_size], F16)
                # alternate the X stream over two queues: doubles arrival
                # rate while keeping per-queue order ~= consumption order
                q = nc.sync if xq[0] % 2 == 0 else nc.scalar
                xq[0] += 1
                q.dma_start(out=x_sb[:], in_=xt16[tb, :, st:st + sz, :])
                x_chunks[(c, tb)] = x_sb

            for tb in range(tb_n):
                for c in range(len(xc_sizes)):
                    x_issue(c, tb)

            # Warm-W stream: blocks 0..3 of every warm panel first (they
            # gate the first chunks), then the rest.
            for jb in range(k_warm):
                w_sb = wpool.tile([P, ib16_n, P], F16, name="w_sb")
                w_tiles.append(w_sb)
            for jb in range(k_warm):
                nc.gpsimd.dma_start(
                    out=w_tiles[jb][:, :4, :], in_=wt16[jb, :, :4, :])
            for jb in range(k_warm):
                nc.gpsimd.dma_start(
                    out=w_tiles[jb][:, 4:, :], in_=wt16[jb, :, 4:, :])

            # Steady prefetch stream (behind the warm-W stream).
            for jb in range(k_warm, w_prefetch):
                w_sb = wpool.tile([P, ib16_n, P], F16, name="w_sb")
                nc.gpsimd.dma_start(out=w_sb[:], in_=wt16[jb])
                w_tiles.append(w_sb)
                if pairs:
                    w8_sb = w8pool.tile([P, pairs, 2, P], F8, name="w8_sb")
                    nc.gpsimd.dma_start(out=w8_sb[:], in_=wt8[jb])
                    w8_tiles.append(w8_sb)

            def x_rhs(ib, tb):
                c, k = ib2chunk[ib]
                return x_chunks[(c, tb)][:, k, :]

            def tbs(tb):
                return slice(tb * tb_size, (tb + 1) * tb_size)

            def evict(ps, jb, tb):
                o_sb = opool.tile([P, tb_size], F32)
                nc.vector.tensor_scalar_add(
                    o_sb[:], ps[:], b_sb[:, jb:jb + 1]
                )
                nc.scalar.dma_start(out=out[jb, :, tbs(tb)], in_=o_sb[:])

            def dr_mms(ps, w8_sb, tb, start, stop):
                for d in range(pairs):
                    nc.tensor.matmul(
                        ps[:], w8_sb[:, d], x8_sbs[tb][:, d],
                        start=(start and d == 0),
                        stop=(stop and d == pairs - 1),
                        perf_mode=mybir.MatmulPerfMode.DoubleRow,
                    )

            # Warm phase: the startup X/W transfer (~12 MiB, ~30+ us) is
            # DMA-bound and longer than one panel's PE work, so the first
            # k_warm panels run chunk-major: each arriving X chunk feeds
            # all warm panels. Every (jb, tb) pair owns one PSUM bank with
            # its own accumulation group; the fp8 DR matmuls run first
            # since their operands are tiny and land first.
            warm_ps = {}
            for tb in range(tb_n):
                for jb in range(k_warm):
                    ps = pspool.tile([P, tb_size], F32, name="ps")
                    warm_ps[(jb, tb)] = ps
                    if pairs:
                        dr_mms(ps, w8_tiles[jb], tb, True, False)
            for tb in range(tb_n):
                for c in range(len(xc_sizes)):
                    st, sz = xc_start[c], xc_sizes[c]
                    last_c = c == len(xc_sizes) - 1
                    for jb in range(k_warm):
                        ps = warm_ps[(jb, tb)]
                        for k in range(sz):
                            nc.tensor.matmul(
                                ps[:],
                                w_tiles[jb][:, st + k, :],
                                x_chunks[(c, tb)][:, k, :],
                                start=(not pairs and c == 0 and k == 0),
                                stop=(last_c and k == sz - 1),
                            )
                        if last_c:
                            evict(ps, jb, tb)

            # Steady phase.
            for jb in range(k_warm, jb_n):
                if jb < w_prefetch:
                    w_sb = w_tiles[jb]
                    w8_sb = w8_tiles[jb] if pairs else None
                else:
                    # W loads ride the gpsimd queue, out DMAs the scalar
                    # queue, keeping the sync queue free.
                    w_sb = wpool.tile([P, ib16_n, P], F16, name="w_sb")
                    nc.gpsimd.dma_start(out=w_sb[:], in_=wt16[jb])
                    if pairs:
                        w8_sb = w8pool.tile([P, pairs, 2, P], F8, name="w8_sb")
                        nc.gpsimd.dma_start(out=w8_sb[:], in_=wt8[jb])
                for tb in range(tb_n):
                    ps = pspool.tile([P, tb_size], F32, name="ps")
                    for ib in range(ib16_n):
                        nc.tensor.matmul(
                            ps[:],
                            w_sb[:, ib, :],
                            x_rhs(ib, tb),
                            start=(ib == 0),
                            stop=(not pairs and ib == ib16_n - 1),
                        )
                    if pairs:
                        dr_mms(ps, w8_sb, tb, False, True)
                    evict(ps, jb, tb)
    nc.compile()
    return nc


_NC_CACHE = {}


def _get_program(ib16_n, pairs):
    key = (ib16_n, pairs)
    if key not in _NC_CACHE:
        _NC_CACHE[key] = build_program(ib16_n=ib16_n, pairs=pairs)
    return _NC_CACHE[key]


def pack_weight(w_sel, ib16_n=IB16_N, pairs=FP8_PAIRS):
    """-> (wt16 [jb,P,ib16,P] f16, wt8 [jb,P,pairs,2,P] e4m3)."""
    # w_sel [jb*P+c, a*P+p] -> [jb, c, a, p]
    w4 = w_sel.reshape(JB_N, P, IB_N, P)
    wt16 = np.ascontiguousarray(
        w4[:, :, :ib16_n, :].transpose(0, 3, 2, 1)).astype(np.float16)
    wt8 = None
    if pairs:
        w8 = w4[:, :, ib16_n:, :].reshape(JB_N, P, pairs, 2, P)
        wt8 = np.ascontiguousarray(
            w8.transpose(0, 4, 2, 3, 1)).astype(NP8)
    return wt16, wt8


def pack_x(data_slice, ib16_n=IB16_N, pairs=FP8_PAIRS, tb_size=TB_SIZE):
    """-> (xt16 [tb_n,P,ib16,tb_size] f16, xt8 [tb_n,P,pairs,2,tb_size])."""
    toks = data_slice.shape[0]
    tb_n = toks // tb_size
    x4 = data_slice.reshape(tb_n, tb_size, IB_N, P)
    xt16 = np.ascontiguousarray(
        x4[:, :, :ib16_n, :].transpose(0, 3, 2, 1)).astype(np.float16)
    xt8 = None
    if pairs:
        x8 = x4[:, :, ib16_n:, :].reshape(tb_n, tb_size, pairs, 2, P)
        xt8 = np.ascontiguousarray(x8.transpose(0, 4, 2, 3, 1)).astype(NP8)
    return xt16, xt8


def pack_bias(b_sel):
    return np.ascontiguousarray(b_sel.reshape(JB_N, P).T.astype(np.float32))


def run(data, weight, bias, mask, trace=False, ib16_n=IB16_N,
        pairs=FP8_PAIRS):
    """Full pipeline; returns (output, BassKernelResults)."""
    data = np.asarray(data, dtype=np.float32)
    weight = np.asarray(weight, dtype=np.float32)
    bias = np.asarray(bias, dtype=np.float32)
    mask = np.asarray(mask)

    # Mirror jnp.nonzero(mask, size=K)[0]: truncate to the first K hits,
    # pad with index 0 when there are fewer than K.
    idx = np.flatnonzero(mask)
    if idx.size >= K_SEL:
        idx = idx[:K_SEL]
    else:
        idx = np.concatenate(
            [idx, np.zeros(K_SEL - idx.size, dtype=idx.dtype)])
    w_sel = weight[idx]
    b_sel = bias[idx]

    wt16_host, wt8_host = pack_weight(w_sel, ib16_n, pairs)
    bs_host = pack_bias(b_sel)

    in_maps = []
    for m in range(N_CORES):
        sl = data[m * TOK_PER_CORE:(m + 1) * TOK_PER_CORE]
        xt16_host, xt8_host = pack_x(sl, ib16_n, pairs)
        im = {"wt16": wt16_host, "xt16": xt16_host, "bs": bs_host}
        if pairs:
            im["wt8"] = wt8_host
            im["xt8"] = xt8_host
        in_maps.append(im)

    nc = _get_program(ib16_n, pairs)

    # Host-side spot check rows (one per device) to detect silent output
    # corruption from transient device faults. Expectation mirrors the
    # device's mixed-precision quantization.
    check_rows = [m * TOK_PER_CORE + (m * 131) % TOK_PER_CORE
                  for m in range(N_CORES)]
    xr = data[check_rows]
    cut = ib16_n * P
    w16 = w_sel[:, :cut].astype(np.float16).astype(np.float32)
    x16 = xr[:, :cut].astype(np.float16).astype(np.float32)
    exp_rows = x16 @ w16.T + b_sel
    if pairs:
        w8 = w_sel[:, cut:].astype(NP8).astype(np.float32)
        x8 = xr[:, cut:].astype(NP8).astype(np.float32)
        exp_rows += x8 @ w8.T
    check_tol = 5e-3 * max(np.abs(exp_rows).max(), 1e-30)

    # Transient NRT/device faults (see trn2 pitfalls: "wedged device") can
    # surface as exceptions OR as corrupted output; validate and retry.
    last_err = None
    for attempt in range(3):
        try:
            res = run_bass_kernel_spmd(
                nc, in_maps, list(range(N_CORES)), trace=trace)
            outT = np.concatenate(
                [r["out"].reshape(K_SEL, TOK_PER_CORE) for r in res.results],
                axis=1,
            )
            got_rows = outT[:, check_rows].T
            err = np.abs(got_rows - exp_rows).max()
            if not np.isfinite(err) or err > check_tol:
                raise RuntimeError(
                    f"device output failed validation (err={err:.3e}, "
                    f"tol={check_tol:.3e}); transient fault suspected")
            return np.ascontiguousarray(outT.T), res
        except Exception as e:  # noqa: BLE001
            last_err = e
            import time as _time
            _time.sleep(5)
    raise last_err


def kernel(data, weight, bias, mask):
    out, _ = run(data, weight, bias, mask)
    return out


# revision 38
# speedup vs baseline: 1.0109x; 1.0109x over previous
"""Dynamic-masked linear (topk_masking) on 8 TRN2 NeuronCores.

Computes reference:
    idx = nonzero(mask)            # exactly K=8192 of 16384
    out = data @ weight[idx].T + bias[idx]     # [8192 tok, 8192 sel]

Strategy (data-parallel over tokens, full selected-weight replicated):
  * Host: nonzero + row-gather of weight/bias (cheap vs 550 GFLOP matmul),
    pack operands into DMA-friendly layouts (partition-major contiguous).
  * Each core m computes out^T[:, m*1024:(m+1)*1024] = W_sel @ X_m^T
    as a PE-stationary-weight matmul over 32 contraction blocks of 128.
  * Mixed precision on the contraction: 28 blocks run as fp16 matmuls
    (1 col/cycle), the last 4 blocks run as 2 fp8-e4m3 DoubleRow matmuls
    (contraction 256 per instr at 1 col/cycle = 2x fp16 throughput,
    measured 220 ns/instr either way). PSUM accumulates fp32. Measured
    full-size rel err ~1.5e-2 (fp8 part dominates; tolerance 2e-2) vs
    3e-4 for pure fp16. 30 instead of 32 PE instrs per output panel.
  * Bias added during PSUM->SBUF eviction via DVE tensor_scalar_add
    (bias is per-partition in the out^T layout).
  * Host: concat the 8 token-slices of out^T, transpose once.

Per-core: 68.7 GFLOP; PE-bound. HBM ~95 MiB (hidden under compute).
"""

import contextlib
import sys
import types
from collections import Counter

import ml_dtypes
import numpy as np

import concourse.bacc as bacc
import concourse.bass as bass
import concourse.mybir as mybir
import concourse.tile as tile
from concourse.bass_utils import run_bass_kernel_spmd


def _ensure_axon_hooks():
    """run_bass_kernel_spmd imports antenv.axon_hooks when tracing is
    requested (e.g. BASS_TRACE=1). Some agent images lack that module;
    provide the real ctypes-based hook when possible, else a None hook so
    tracing degrades gracefully instead of crashing the kernel."""
    if "antenv.axon_hooks" in sys.modules:
        return
    try:
        import antenv.axon_hooks  # noqa: F401
        return
    except ImportError:
        pass
    hook = None
    try:
        from trn_agent_boot.trn_boot import _ntff_profile_via_ctypes
        hook = _ntff_profile_via_ctypes("/opt/axon/libaxon_pjrt.so")
    except Exception:
        pass
    mod = types.ModuleType("antenv.axon_hooks")
    mod.get_axon_ntff_profile_hook = lambda: hook
    mod.set_axon_ntff_profile_hook = lambda h: None
    sys.modules["antenv.axon_hooks"] = mod


_ensure_axon_hooks()

N_CORES = 8
P = 128

# Full-problem dims (hardcoded per harness contract)
IN_F = 4096
OUT_F = 16384
N_TOK = 8192
K_SEL = OUT_F // 2
TOK_PER_CORE = N_TOK // N_CORES  # 1024

IB_N = IN_F // P    # 32 contraction blocks
JB_N = K_SEL // P   # 64 output-column panels
TB_SIZE = 512       # moving free dim per matmul (one PSUM bank of fp32)

# Contraction split: first IB16_N blocks in fp16, last 4 in fp8 e4m3
# (2 DoubleRow pairs). More fp8 blows the 2e-2 error budget: full-fp8
# rel err is 4.4e-2 and scales with sqrt(n_fp8/32).
FP8_PAIRS = 2
IB8_N = 2 * FP8_PAIRS
IB16_N = IB_N - IB8_N

F32 = mybir.dt.float32
F16 = mybir.dt.float16
F8 = mybir.dt.float8e4
NP8 = ml_dtypes.float8_e4m3


def build_program(ib16_n=IB16_N, pairs=FP8_PAIRS, toks=TOK_PER_CORE,
                  tb_size=TB_SIZE, jb_n=JB_N, w_bufs=6):
    """Build the per-core Bass program.

    DRAM parameter layouts (host packs these):
      wt16 [jb_n, P, ib16_n, P]  : wt16[jb, p, a, c] = f16(W_sel[jb*P+c, a*P+p])
      wt8  [jb_n, P, pairs, 2, P]: wt8[jb, p, d, i, c]
                                     = e4m3(W_sel[jb*P+c, (ib16_n+2d+i)*P+p])
      xt16 [tb_n, P, ib16_n, tb] : xt16[tb, p, a, t]
                                     = f16(data[tok0+tb*tb_size+t, a*P+p])
      xt8  [P, pairs, 2, toks]   : xt8[p, d, i, t]
                                     = e4m3(data[tok0+t, (ib16_n+2d+i)*P+p])
      bs   [P, jb_n]             : bs[c, jb] = b_sel[jb*P + c]
      out  [jb_n, P, toks]       : out[jb, c, t] = out^T[jb*P+c, tok0+t]
    """
    tb_n = toks // tb_size
    assert toks % tb_size == 0

    nc = bacc.Bacc(
        "TRN2", target_bir_lowering=False, debug=False, num_devices=N_CORES
    )
    wt16 = nc.declare_dram_parameter(
        "wt16", [jb_n, P, ib16_n, P], F16, isOutput=False)
    xt16 = nc.declare_dram_parameter(
        "xt16", [tb_n, P, ib16_n, tb_size], F16, isOutput=False)
    if pairs:
        wt8 = nc.declare_dram_parameter(
            "wt8", [jb_n, P, pairs, 2, P], F8, isOutput=False)
        xt8 = nc.declare_dram_parameter(
            "xt8", [tb_n, P, pairs, 2, tb_size], F8, isOutput=False)
    bs = nc.declare_dram_parameter("bs", [P, jb_n], F32, isOutput=False)
    out = nc.declare_dram_parameter("out", [jb_n, P, toks], F32, isOutput=True)

    # x16-load chunking: few DMA triggers (issue is ~650 ns each per queue),
    # ramping sizes so the first matmuls unblock as early as possible.
    xc_sizes = [1, 1, 2, 4]
    while sum(xc_sizes) < ib16_n:
        xc_sizes.append(min(8, ib16_n - sum(xc_sizes)))
    xc_start = np.cumsum([0] + xc_sizes)
    ib2chunk = {}
    for c, (st, sz) in enumerate(zip(xc_start, xc_sizes)):
        for k in range(sz):
            ib2chunk[st + k] = (c, k)

    size_counts = Counter(xc_sizes)

    with tile.TileContext(nc) as tc:
        with contextlib.ExitStack() as stk:
            xpools = {
                sz: stk.enter_context(
                    tc.tile_pool(name=f"xpool{sz}", bufs=cnt * tb_n))
                for sz, cnt in size_counts.items()
            }
            wpool = stk.enter_context(tc.tile_pool(name="wpool", bufs=w_bufs))
            if pairs:
                x8pool = stk.enter_context(
                    tc.tile_pool(name="x8pool", bufs=tb_n))
                w8pool = stk.enter_context(
                    tc.tile_pool(name="w8pool", bufs=w_bufs))
            bpool = stk.enter_context(tc.tile_pool(name="bpool", bufs=1))
            opool = stk.enter_context(tc.tile_pool(name="opool", bufs=6))
            pspool = stk.enter_context(
                tc.tile_pool(name="pspool", bufs=8, space="PSUM"))

            # Startup DMAs are issued in CONSUMPTION order, round-robin
            # over the three trigger queues. Each queue preserves its own
            # order (ring-blocking), so a large transfer only delays every
            # third item; chunk-sized slices keep the skew small.
            k_warm = min(4, jb_n)
            w_prefetch = min(w_bufs, jb_n)
            w_tiles = []
            w8_tiles = []
            queues = [nc.sync, nc.gpsimd, nc.scalar]
            qi = [0]

            def issue(out_ap, in_ap):
                queues[qi[0] % len(queues)].dma_start(out=out_ap, in_=in_ap)
                qi[0] += 1

            b_sb = bpool.tile([P, jb_n], F32)
            issue(b_sb[:], bs[:])
            x8_sbs = []
            if pairs:
                for tb in range(tb_n):
                    x8_sb = x8pool.tile(
                        [P, pairs, 2, tb_size], F8, name="x8_sb")
                    issue(x8_sb[:], xt8[tb])
                    x8_sbs.append(x8_sb)
                for jb in range(k_warm):
                    w8_sb = w8pool.tile([P, pairs, 2, P], F8, name="w8_sb")
                    issue(w8_sb[:], wt8[jb])
                    w8_tiles.append(w8_sb)

            x_chunks = {}

            def x_issue(c, tb):
                st, sz = xc_start[c], xc_sizes[c]
                x_sb = xpools[sz].tile([P, sz, tb_size], F16)
                issue(x_sb[:], xt16[tb, :, st:st + sz, :])
                x_chunks[(c, tb)] = x_sb

            # Warm W panels stream in consumption-sized slices interleaved
            # with the X chunk ramp.
            for jb in range(k_warm):
                w_sb = wpool.tile([P, ib16_n, P], F16, name="w_sb")
                w_tiles.append(w_sb)
            for jb in range(k_warm):
                issue(w_tiles[jb][:, :4, :], wt16[jb, :, :4, :])
            x_issue(0, 0)
            x_issue(1, 0)
            x_issue(2, 0)
            for jb in range(k_warm):
                issue(w_tiles[jb][:, 4:16, :], wt16[jb, :, 4:16, :])
            x_issue(3, 0)
            for jb in range(k_warm):
                issue(w_tiles[jb][:, 16:, :], wt16[jb, :, 16:, :])
            for c in range(4, len(xc_sizes)):
                x_issue(c, 0)
            for tb in range(1, tb_n):
                for c in range(len(xc_sizes)):
                    x_issue(c, tb)

            # Steady prefetch stream (after everything warm).
            for jb in range(k_warm, w_prefetch):
                w_sb = wpool.tile([P, ib16_n, P], F16, name="w_sb")
                issue(w_sb[:], wt16[jb])
                w_tiles.append(w_sb)
                if pairs:
                    w8_sb = w8pool.tile([P, pairs, 2, P], F8, name="w8_sb")
                    issue(w8_sb[:], wt8[jb])
                    w8_tiles.append(w8_sb)

            def x_rhs(ib, tb):
                c, k = ib2chunk[ib]
                return x_chunks[(c, tb)][:, k, :]

            def tbs(tb):
                return slice(tb * tb_size, (tb + 1) * tb_size)

            def evict(ps, jb, tb):
                o_sb = opool.tile([P, tb_size], F32)
                nc.vector.tensor_scalar_add(
                    o_sb[:], ps[:], b_sb[:, jb:jb + 1]
                )
                nc.scalar.dma_start(out=out[jb, :, tbs(tb)], in_=o_sb[:])

            def dr_mms(ps, w8_sb, tb, start, stop):
                for d in range(pairs):
                    nc.tensor.matmul(
                        ps[:], w8_sb[:, d], x8_sbs[tb][:, d],
                        start=(start and d == 0),
                        stop=(stop and d == pairs - 1),
                        perf_mode=mybir.MatmulPerfMode.DoubleRow,
                    )

            # Warm phase: the startup X/W transfer (~12 MiB, ~30+ us) is
            # DMA-bound and longer than one panel's PE work, so the first
            # k_warm panels run chunk-major: each arriving X chunk feeds
            # all warm panels. Every (jb, tb) pair owns one PSUM bank with
            # its own accumulation group; the fp8 DR matmuls run first
            # since their operands are tiny and land first.
            warm_ps = {}
            for tb in range(tb_n):
                for jb in range(k_warm):
                    ps = pspool.tile([P, tb_size], F32, name="ps")
                    warm_ps[(jb, tb)] = ps
                    if pairs:
                        dr_mms(ps, w8_tiles[jb], tb, True, False)
            for tb in range(tb_n):
                for c in range(len(xc_sizes)):
                    st, sz = xc_start[c], xc_sizes[c]
                    last_c = c == len(xc_sizes) - 1
                    for jb in range(k_warm):
                        ps = warm_ps[(jb, tb)]
                        for k in range(sz):
                            nc.tensor.matmul(
                                ps[:],
                                w_tiles[jb][:, st + k, :],
                                x_chunks[(c, tb)][:, k, :],
                                start=(not pairs and c == 0 and k == 0),
                                stop=(last_c and k == sz - 1),
                            )
                        if last_c:
                            evict(ps, jb, tb)

            # Steady phase.
            for jb in range(k_warm, jb_n):
                if jb < w_prefetch:
                    w_sb = w_tiles[jb]
                    w8_sb = w8_tiles[jb] if pairs else None
                else:
                    # W loads ride the gpsimd queue, out DMAs the scalar
                    # queue, keeping the sync queue free.
                    w_sb = wpool.tile([P, ib16_n, P], F16, name="w_sb")
                    nc.gpsimd.dma_start(out=w_sb[:], in_=wt16[jb])
                    if pairs:
                        w8_sb = w8pool.tile([P, pairs, 2, P], F8, name="w8_sb")
                        nc.gpsimd.dma_start(out=w8_sb[:], in_=wt8[jb])
                for tb in range(tb_n):
                    ps = pspool.tile([P, tb_size], F32, name="ps")
                    for ib in range(ib16_n):
                        nc.tensor.matmul(
                            ps[:],
                            w_sb[:, ib, :],
                            x_rhs(ib, tb),
                            start=(ib == 0),
                            stop=(not pairs and ib == ib16_n - 1),
                        )
                    if pairs:
                        dr_mms(ps, w8_sb, tb, False, True)
                    evict(ps, jb, tb)
    nc.compile()
    return nc


_NC_CACHE = {}


def _get_program(ib16_n, pairs):
    key = (ib16_n, pairs)
    if key not in _NC_CACHE:
        _NC_CACHE[key] = build_program(ib16_n=ib16_n, pairs=pairs)
    return _NC_CACHE[key]


def pack_weight(w_sel, ib16_n=IB16_N, pairs=FP8_PAIRS):
    """-> (wt16 [jb,P,ib16,P] f16, wt8 [jb,P,pairs,2,P] e4m3)."""
    # w_sel [jb*P+c, a*P+p] -> [jb, c, a, p]
    w4 = w_sel.reshape(JB_N, P, IB_N, P)
    wt16 = np.ascontiguousarray(
        w4[:, :, :ib16_n, :].transpose(0, 3, 2, 1)).astype(np.float16)
    wt8 = None
    if pairs:
        w8 = w4[:, :, ib16_n:, :].reshape(JB_N, P, pairs, 2, P)
        wt8 = np.ascontiguousarray(
            w8.transpose(0, 4, 2, 3, 1)).astype(NP8)
    return wt16, wt8


def pack_x(data_slice, ib16_n=IB16_N, pairs=FP8_PAIRS, tb_size=TB_SIZE):
    """-> (xt16 [tb_n,P,ib16,tb_size] f16, xt8 [tb_n,P,pairs,2,tb_size])."""
    toks = data_slice.shape[0]
    tb_n = toks // tb_size
    x4 = data_slice.reshape(tb_n, tb_size, IB_N, P)
    xt16 = np.ascontiguousarray(
        x4[:, :, :ib16_n, :].transpose(0, 3, 2, 1)).astype(np.float16)
    xt8 = None
    if pairs:
        x8 = x4[:, :, ib16_n:, :].reshape(tb_n, tb_size, pairs, 2, P)
        xt8 = np.ascontiguousarray(x8.transpose(0, 4, 2, 3, 1)).astype(NP8)
    return xt16, xt8


def pack_bias(b_sel):
    return np.ascontiguousarray(b_sel.reshape(JB_N, P).T.astype(np.float32))


def run(data, weight, bias, mask, trace=False, ib16_n=IB16_N,
        pairs=FP8_PAIRS):
    """Full pipeline; returns (output, BassKernelResults)."""
    data = np.asarray(data, dtype=np.float32)
    weight = np.asarray(weight, dtype=np.float32)
    bias = np.asarray(bias, dtype=np.float32)
    mask = np.asarray(mask)

    # Mirror jnp.nonzero(mask, size=K)[0]: truncate to the first K hits,
    # pad with index 0 when there are fewer than K.
    idx = np.flatnonzero(mask)
    if idx.size >= K_SEL:
        idx = idx[:K_SEL]
    else:
        idx = np.concatenate(
            [idx, np.zeros(K_SEL - idx.size, dtype=idx.dtype)])
    w_sel = weight[idx]
    b_sel = bias[idx]

    wt16_host, wt8_host = pack_weight(w_sel, ib16_n, pairs)
    bs_host = pack_bias(b_sel)

    in_maps = []
    for m in range(N_CORES):
        sl = data[m * TOK_PER_CORE:(m + 1) * TOK_PER_CORE]
        xt16_host, xt8_host = pack_x(sl, ib16_n, pairs)
        im = {"wt16": wt16_host, "xt16": xt16_host, "bs": bs_host}
        if pairs:
            im["wt8"] = wt8_host
            im["xt8"] = xt8_host
        in_maps.append(im)

    nc = _get_program(ib16_n, pairs)

    # Host-side spot check rows (one per device) to detect silent output
    # corruption from transient device faults. Expectation mirrors the
    # device's mixed-precision quantization.
    check_rows = [m * TOK_PER_CORE + (m * 131) % TOK_PER_CORE
                  for m in range(N_CORES)]
    xr = data[check_rows]
    cut = ib16_n * P
    w16 = w_sel[:, :cut].astype(np.float16).astype(np.float32)
    x16 = xr[:, :cut].astype(np.float16).astype(np.float32)
    exp_rows = x16 @ w16.T + b_sel
    if pairs:
        w8 = w_sel[:, cut:].astype(NP8).astype(np.float32)
        x8 = xr[:, cut:].astype(NP8).astype(np.float32)
        exp_rows += x8 @ w8.T
    check_tol = 5e-3 * max(np.abs(exp_rows).max(), 1e-30)

    # Transient NRT/device faults (see trn2 pitfalls: "wedged device") can
    # surface as exceptions OR as corrupted output; validate and retry.
    last_err = None
    for attempt in range(3):
        try:
            res = run_bass_kernel_spmd(
                nc, in_maps, list(range(N_CORES)), trace=trace)
            outT = np.concatenate(
                [r["out"].reshape(K_SEL, TOK_PER_CORE) for r in res.results],
                axis=1,
            )
            got_rows = outT[:, check_rows].T
            err = np.abs(got_rows - exp_rows).max()
            if not np.isfinite(err) or err > check_tol:
                raise RuntimeError(
                    f"device output failed validation (err={err:.3e}, "
                    f"tol={check_tol:.3e}); transient fault suspected")
            return np.ascontiguousarray(outT.T), res
        except Exception as e:  # noqa: BLE001
            last_err = e
            import time as _time
            _time.sleep(5)
    raise last_err


def kernel(data, weight, bias, mask):
    out, _ = run(data, weight, bias, mask)
    return out
